# revision 8
# baseline (speedup 1.0000x reference)
"""Trainium2 Bass kernel for a contrastive (hinge) loss.

loss = (1/B) * sum_{i,j != t_i} relu(1 - ||f_i - c_j||^2)

Strategy (data-parallel over 8 NeuronCores, batch sharded, class table
replicated):
  dist[i,j] = f2[i] + c2[j] - 2*cross[i,j]
  hinge     = relu(1 - dist) = 2*(max(cross + beta[i], gamma[j]) - gamma[j])
  with beta = (1-f2)/2, gamma = c2/2.

Per core (2048 rows = 16 tiles of 128 partitions):
  - cross tiles [128,1000] via PE matmul in fp16 (F^T tiles x C^T), built
    with one DMA-transpose instruction each for F^T and C^T.
  - ACT-route tiles: PE rank-1 accumulates -gamma[j] into PSUM, then one
    ScalarE Relu(x+beta) pass with fused row-sum accumulation (exact +0.0
    when no hinge is active).
  - DVE-route tiles: one VectorE scalar_tensor_tensor pass
    max(x+beta, gamma_bcast) with fused row-sum; a bitwise-matched
    calibration row-sum acc0 = sum_j gamma[j] (same instruction shape) is
    subtracted per row so inactive tiles contribute exactly 0.0.
  - target term (j == t_i) is subtracted via relu(dot + beta - gamma_t):
    class rows gathered by indirect DMA, dot via row-wise multiply+reduce,
    gamma_t gathered from a small DRAM bounce of the gamma vector.
  - final partition reduction via a PE matmul with ones; scaled by 2/B.
Host sums the 8 per-core partials (each already scaled by 2/B).
"""

import numpy as np

B, C, D = 16384, 1000, 128
NCORES = 8
BS = B // NCORES          # 2048 rows per core
NT = BS // 128            # 16 batch tiles per core
CPAD = 1024               # class dim padded to 8*128
N_ACT = 9                 # tiles on the ScalarE route (t < N_ACT); rest on DVE

_CACHE = {}


def _build_nc():
    if "nc" in _CACHE:
        return _CACHE["nc"]

    from contextlib import ExitStack

    import concourse.bacc as bacc
    import concourse.bass as bass
    import concourse.mybir as mybir
    import concourse.tile as tile
    from concourse.tile import add_dep_helper

    dt = mybir.dt
    AF = mybir.ActivationFunctionType
    ALU = mybir.AluOpType
    AX = mybir.AxisListType

    nc = bacc.Bacc(
        "TRN2", target_bir_lowering=False, debug=False, num_devices=NCORES
    )

    feat = nc.dram_tensor("feat", [BS, D], dt.float32, kind="ExternalInput")
    cls = nc.dram_tensor("cls", [C, D], dt.float32, kind="ExternalInput")
    tgt = nc.dram_tensor("tgt", [128, NT], dt.int32, kind="ExternalInput")
    out = nc.dram_tensor("out", [1, 1], dt.float32, kind="ExternalOutput")

    with tile.TileContext(nc) as tc, ExitStack() as ctx:
        sing = ctx.enter_context(tc.tile_pool(name="sing", bufs=1))
        psp = ctx.enter_context(tc.tile_pool(name="psp", bufs=4, space="PSUM"))
        dramp = ctx.enter_context(tc.tile_pool(name="dramp", bufs=1, space="DRAM"))

        F32 = sing.tile([128, NT, 128], dt.float32)
        F16 = sing.tile([128, NT, 128], dt.float16)
        FT = sing.tile([128, NT, 128], dt.float16)
        C32 = sing.tile([128, 8, 128], dt.float32)
        C16 = sing.tile([128, 8, 128], dt.float16)
        CT = sing.tile([128, 8, 128], dt.float16)
        CTSQ = sing.tile([128, CPAD], dt.float16)
        GB = sing.tile([128, CPAD], dt.float32)
        SQ = sing.tile([128, NT, 128], dt.float16)
        CTA = sing.tile([128, NT, 128], dt.float32)
        QQ = sing.tile([128, NT, 128], dt.float16)
        grow = sing.tile([1, CPAD], dt.float16)
        ones_col = sing.tile([128, 1], dt.float16)
        negones = sing.tile([1, 128], dt.float16)
        posones = sing.tile([1, 128], dt.float16)
        ones_red = sing.tile([128, 1], dt.float32)
        tgt_sb = sing.tile([128, NT], dt.int32)
        acc = sing.tile([128, NT], dt.float32)
        acc0 = sing.tile([128, 1], dt.float32)
        negbig = sing.tile([128, 1], dt.float32)
        f2 = sing.tile([128, NT], dt.float16)
        beta = sing.tile([128, NT], dt.float32)
        dotc = sing.tile([128, NT], dt.float16)
        gt = sing.tile([128, NT], dt.float16)
        s_all = sing.tile([128, NT], dt.float32)
        m_all = sing.tile([128, NT], dt.float32)
        corr = sing.tile([128, NT], dt.float32)
        tot = sing.tile([128, NT], dt.float32)
        vcol = sing.tile([128, 1], dt.float32)
        out_sb = sing.tile([1, 1], dt.float32)
        gdram = dramp.tile([1, CPAD], dt.float16)

        # ---- loads (class path first: it heads the longest dep chain)
        nc.sync.dma_start(out=tgt_sb[:, :], in_=tgt.ap())
        nc.sync.dma_start(
            out=C32[:, 0:7, :],
            in_=cls.ap()[0:896, :].rearrange("(c p) d -> p c d", p=128),
        )
        nc.gpsimd.memset(C32[:, 7, :], 0.0)
        nc.sync.dma_start(out=C32[0:104, 7, :], in_=cls.ap()[896:1000, :])
        nc.sync.dma_start(
            out=F32[:, :, :],
            in_=feat.ap().rearrange("(t p) d -> p t d", p=128),
        )

        # gather target class rows early (independent long-running DMA)
        nc.gpsimd.indirect_dma_start(
            out=CTA[:, :, :],
            out_offset=None,
            in_=cls.ap(),
            in_offset=bass.IndirectOffsetOnAxis(ap=tgt_sb[:, :], axis=0),
        )

        # ---- fp16 casts + DMA block-transposes
        nc.gpsimd.tensor_copy(out=C16[:, :, :], in_=C32[:, :, :])
        nc.sync.dma_start_transpose(out=CT[:, :, :], in_=C16[:, :, :])
        nc.vector.tensor_copy(out=F16[:, :, :], in_=F32[:, :, :])
        nc.sync.dma_start_transpose(out=FT[:, :, :], in_=F16[:, :, :])

        ct_rhs = CT[:, :, :].rearrange("p a b -> p (a b)")  # [128, 1024] fp16

        # ---- constants
        nc.vector.memset(ones_col[:, :], 1.0)
        nc.vector.memset(negones[:, :], -1.0)
        nc.vector.memset(posones[:, :], 1.0)
        nc.vector.memset(ones_red[:, :], 1.0)
        nc.vector.memset(negbig[:, :], -1e30)

        # ---- gamma chain: c2 = sum_d C^2 via ones^T @ (CT*CT)
        nc.gpsimd.tensor_mul(CTSQ[:, :], ct_rhs, ct_rhs)
        c2ps = psp.tile([128, CPAD], dt.float32, tag="ps")
        nc.tensor.matmul(
            out=c2ps[0:1, 0:512], lhsT=ones_col[:, :], rhs=CTSQ[:, 0:512],
            start=True, stop=True,
        )
        nc.tensor.matmul(
            out=c2ps[0:1, 512:1024], lhsT=ones_col[:, :], rhs=CTSQ[:, 512:1024],
            start=True, stop=True,
        )
        # gamma row (+0.5*c2) on partition 0
        nc.scalar.activation(
            out=grow[0:1, :], in_=c2ps[0:1, 0:1024], func=AF.Copy,
            bias=0.0, scale=0.5,
        )
        # bounce gamma to DRAM, gather gamma[t_i] per row
        st = nc.gpsimd.dma_start(out=gdram[0:1, :], in_=grow[0:1, :])
        gi = nc.gpsimd.indirect_dma_start(
            out=gt[:, :],
            out_offset=None,
            in_=gdram[0:1, :],
            in_offset=bass.IndirectOffsetOnAxis(ap=tgt_sb[:, :], axis=1),
        )
        add_dep_helper(gi.ins, st.ins, reason="gamma store before gather")

        # broadcast +gamma to all partitions via rank-1 matmul
        gbps = psp.tile([128, CPAD], dt.float32, tag="ps")
        nc.tensor.matmul(
            out=gbps[:, 0:512],
            lhsT=posones[0:1, :],
            rhs=grow[0:1, 0:512],
            start=True, stop=True,
        )
        nc.tensor.matmul(
            out=gbps[:, 512:1024],
            lhsT=posones[0:1, :],
            rhs=grow[0:1, 512:1024],
            start=True, stop=True,
        )
        nc.scalar.activation(
            out=GB[:, :], in_=gbps[:, 0:1024], func=AF.Copy, bias=0.0, scale=1.0
        )
        # calibration row-sum: bitwise-identical accumulation of sum_j gamma
        nc.vector.scalar_tensor_tensor(
            out=gbps[:, 0:1000], in0=gbps[:, 0:1000], scalar=negbig[:, :],
            in1=GB[:, 0:1000], op0=ALU.add, op1=ALU.max, accum_out=acc0[:, :],
        )

        # ---- f2 = sum_d F^2, beta = (1 - f2)/2
        f16_flat = F16[:, :, :].rearrange("p a b -> p (a b)")
        sq_flat = SQ[:, :, :].rearrange("p a b -> p (a b)")
        nc.scalar.activation(
            out=sq_flat, in_=f16_flat, func=AF.Square, bias=0.0, scale=1.0
        )
        with nc.allow_low_precision(reason="f2 in fp16 is plenty for a hinge threshold"):
            nc.vector.tensor_reduce(
                out=f2[:, :], in_=SQ[:, :, :], axis=AX.X, op=ALU.add
            )
        nc.vector.tensor_scalar(beta[:, :], f2[:, :], -0.5, 0.5, ALU.mult, ALU.add)

        # ---- main loop over batch tiles
        for t in range(NT):
            ps = psp.tile([128, CPAD], dt.float32, tag="ps")
            lhs = FT[:, t, :]
            is_act = t < N_ACT
            nc.tensor.matmul(
                out=ps[:, 0:512], lhsT=lhs, rhs=ct_rhs[:, 0:512],
                start=True, stop=not is_act,
            )
            nc.tensor.matmul(
                out=ps[:, 512:1000], lhsT=lhs, rhs=ct_rhs[:, 512:1000],
                start=True, stop=not is_act,
            )
            if is_act:
                nc.tensor.matmul(
                    out=ps[:, 0:512],
                    lhsT=negones[0:1, :],
                    rhs=grow[0:1, 0:512],
                    start=False, stop=True,
                )
                nc.tensor.matmul(
                    out=ps[:, 512:1000],
                    lhsT=negones[0:1, :],
                    rhs=grow[0:1, 512:1000],
                    start=False, stop=True,
                )
                nc.scalar.activation(
                    out=ps[:, 0:1000], in_=ps[:, 0:1000], func=AF.Relu,
                    bias=beta[:, t:t + 1], scale=1.0,
                    accum_out=acc[:, t:t + 1],
                )
            else:
                nc.vector.scalar_tensor_tensor(
                    out=ps[:, 0:1000], in0=ps[:, 0:1000],
                    scalar=beta[:, t:t + 1], in1=GB[:, 0:1000],
                    op0=ALU.add, op1=ALU.max, accum_out=acc[:, t:t + 1],
                )

        # ---- target term: dot = sum_d F*c_t per row
        f32_flat = F32[:, :, :].rearrange("p a b -> p (a b)")
        cta_flat = CTA[:, :, :].rearrange("p a b -> p (a b)")
        qq_flat = QQ[:, :, :].rearrange("p a b -> p (a b)")
        nc.gpsimd.tensor_mul(qq_flat, f32_flat, cta_flat)
        with nc.allow_low_precision(reason="target-dot fp16 tolerance is ample"):
            nc.vector.tensor_reduce(
                out=dotc[:, :], in_=QQ[:, :, :], axis=AX.X, op=ALU.add
            )
        nc.vector.tensor_add(s_all[:, :], dotc[:, :], beta[:, :])
        nc.vector.tensor_max(m_all[:, :], s_all[:, :], gt[:, :])
        nc.vector.tensor_sub(corr[:, :], m_all[:, :], gt[:, :])

        # ---- combine: subtract calibration from DVE tiles, then reduce
        nc.vector.tensor_scalar(
            acc[:, N_ACT:NT], acc[:, N_ACT:NT], acc0[:, :], None, ALU.subtract
        )
        nc.vector.tensor_sub(tot[:, :], acc[:, :], corr[:, :])
        nc.vector.tensor_reduce(out=vcol[:, :], in_=tot[:, :], axis=AX.X, op=ALU.add)
        fps = psp.tile([128, CPAD], dt.float32, tag="ps")
        nc.tensor.matmul(
            out=fps[0:1, 0:1], lhsT=vcol[:, :], rhs=ones_red[:, :],
            start=True, stop=True,
        )
        nc.scalar.activation(
            out=out_sb[:, :], in_=fps[0:1, 0:1], func=AF.Copy,
            bias=0.0, scale=2.0 / float(B),
        )
        nc.sync.dma_start(out=out.ap(), in_=out_sb[:, :])

    nc.compile()
    _CACHE["nc"] = nc
    return nc


def kernel(features, targets, class_feature_vectors):
    nc = _build_nc()
    from concourse.bass_utils import run_bass_kernel_spmd

    f = np.ascontiguousarray(np.asarray(features, dtype=np.float32))
    t = np.asarray(targets).astype(np.int32)
    c = np.ascontiguousarray(np.asarray(class_feature_vectors, dtype=np.float32))
    assert f.shape == (B, D) and c.shape == (C, D) and t.shape == (B,)

    in_maps = []
    for k in range(NCORES):
        fs = f[k * BS:(k + 1) * BS]
        ts = np.ascontiguousarray(t[k * BS:(k + 1) * BS].reshape(NT, 128).T)
        in_maps.append({"feat": fs, "cls": c, "tgt": ts})

    res = run_bass_kernel_spmd(nc, in_maps, core_ids=list(range(NCORES)))
    parts = [r["out"][0, 0] for r in res.results]
    total = np.float32(np.sum(np.asarray(parts, dtype=np.float64)))
    return np.array(total, dtype=np.float32)
